# revision 42
# baseline (speedup 1.0000x reference)
"""Trainium2 Bass kernel for a binarized (1w1a) BasicBlock:

    out = BN2(PReLU(conv3x3(sign(x1), std2*sign(W2)) + b2)) + x1
    x1  = BN1(PReLU(conv3x3(sign(x),  std1*sign(W1)) + b1)) + x

Strategy
--------
Data-parallel over the batch axis: each of the 8 NeuronCores processes 8 of
the 64 images, with the (small) weights / BN / PReLU params replicated.
No collectives are needed.

Per-core compute:
  * Activations and weights are binarized (ScalarE Sign -> {-1,0,+1} for
    conv1's input; the (x>0)-0.5 = +-0.5 VectorE trick for weights and for
    sign(out1)), exactly representable in fp8e4m3.  The 3x3 conv over
    256->256 channels is 9 shifted matmuls accumulating in PSUM, using the
    fp8 DoubleRow perf mode so each matmul contracts the full K=256 input
    channels (2 fp8 weights per PE cell).  The scale factors (x2 conv1,
    x4 conv2) are folded into the BN scale of the PSUM evacuation.
  * Pad-row-free strip layout: per partition, super-row r (r = 0..31)
    concatenates row r of all 8 images, each 33 wide (32 real + 1 zero pad
    col); 32 super-rows of 264 form one 8448-element strip per channel
    half, with zero guards on both ends.  Vertical 3x3 taps are +-264,
    horizontal +-1, so every tap is a constant offset and a 396-element
    window (<= 1 PSUM bank) is one uniform-stride matmul.
  * The PE stream (~131us) is the spine; everything else hides behind it:
    - PE warm-up: dummy matmuls on an all-ones fp8 tile while the first
      DMAs land, so HAM un-throttles (1.2 -> 2.4 GHz) before the first
      real matmul instead of ~5us into the stream.
    - PSUM evacuation is ONE ScalarE op: AFT.Prelu with per-channel alpha
      fuses conv-bias + BN scale + PReLU (PReLU commutes with the
      positive gamma scale).  DVE then does one fused add (+ BN shift
      via the scalar operand, + residual) per window.
    - sign(out1) runs on DVE as (o1>0)-0.5 (ScalarE's FIFO is the
      contended resource early on; the 2x folds into conv2's scale).
    - x feeds: a host-cast bf16 copy of x feeds the sign path (half the
      startup-critical bytes; bf16 is sign-exact), the f32 copy loads
      later for the residual adds only.  One DMA per pair, params table
      first on the ring, w1 in {taps 0-4}/{taps 5-8} chunks interleaved
      at need-order ring slots.  w2's staging is dep-pinned behind o1p[0]
      so the (ready-list) Tile scheduler cannot hoist its bytes into the
      startup window, and 3-tap-chunked so a late DMA never head-blocks
      the DVE queue.
    - conv groups: singleton first group (first matmuls gate only on pair
      0) and singleton last groups (short final post-op chain); per-pair
      [128,2,ln] output stores alternate between the Sync HWDGE ring and
      the (idle by then) GpSimd SWDGE ring.
  * Writes into the s1/s2 strips stay contiguous-per-half where matmuls
    must not over-wait: the Tile range tracker coarsens strided writes to
    whole-tile deps, which serializes the PE behind unrelated sign ops.

The host side only reshapes/transposes/zero-pads/casts (layout), shards the
batch and un-packs the output strip.  All arithmetic (sign, BN folding,
conv, PReLU, residual) happens on-device.
"""

import math
import os
import sys

import numpy as np

for _p in ("/opt/trn_rl_repo", "/root/.axon_site/_ro/trn_rl_repo"):
    if os.path.isdir(_p) and _p not in sys.path:
        sys.path.insert(0, _p)

import concourse.bass as bass
import concourse.bacc as bacc
import concourse.mybir as mybir
from concourse import tile
from concourse.bass_utils import run_bass_kernel_spmd

F32 = mybir.dt.float32
BF16 = mybir.dt.bfloat16
F8 = mybir.dt.float8e4
AOP = mybir.AluOpType
AFT = mybir.ActivationFunctionType
DR = mybir.MatmulPerfMode.DoubleRow

EPS = 1e-5
NCORES = 8
NIMG = 8            # images per core
NCOL = 33           # cols per image row (32 real + 1 pad)
SR = NIMG * NCOL    # super-row length: row r of all 8 images  (264)
NSR = 32            # super-rows per strip
STRIP = NSR * SR    # 8448
GF = 272            # front guard (>= SR + 1, 16-aligned)
WCOLS = 396         # window: 1.5 super-rows (<= 512 PSUM bank)
STD = math.sqrt(2.0) / math.sqrt(256 * 9)

# strip tile splits, in super-rows (see module docstring)
S1CUTS = [(0, 18), (15, 32)]
S2CUTS = [(0, 18), (15, 25), (23, 32)]
W2T_1 = [0] * 11 + [1] * 11              # window -> s1 tile
W2T_2 = [0] * 11 + [1] * 5 + [2] * 6     # window -> s2 tile


def _slen(cut):
    n = GF + (cut[1] - cut[0]) * SR + 288
    return ((n + 15) // 16) * 16


S1LENS = [_slen(c) for c in S1CUTS]
S2LENS = [_slen(c) for c in S2CUTS]


def route(cuts, ra, rb):
    """Map super-row range [ra, rb) onto the strip tiles."""
    out = []
    for t, (lo, hi) in enumerate(cuts):
        a, b = max(ra, lo), min(rb, hi)
        if a < b:
            out.append((t, a, b))
    return out


# windows: (e0, ln); 21 full 396-col windows + one 132-col tail
WINDOWS = []
_e = 0
while _e < STRIP:
    ln = min(WCOLS, STRIP - _e)
    WINDOWS.append((_e, ln))
    _e += ln

# pairs: 2 windows each -> super-row aligned (3 rows; last pair 2 rows)
PAIRS = []
for _i in range(0, len(WINDOWS), 2):
    ws_ = WINDOWS[_i:_i + 2]
    e0 = ws_[0][0]
    ln = sum(w[1] for w in ws_)
    assert e0 % SR == 0 and ln % SR == 0
    PAIRS.append((list(range(_i, _i + len(ws_))), e0, ln,
                  e0 // SR, (e0 + ln) // SR))

# per-channel param column order inside the packed [128, 22] table
PARAM_ORDER = [
    "b1", "alpha", "bn1_gamma", "bn1_beta", "bn1_mean", "bn1_var",
    "b2", "bn2_gamma", "bn2_beta", "bn2_mean", "bn2_var",
]
NPARAM = len(PARAM_ORDER)


def _rows_ap(t2d, a, b, base=0):
    """[128, b-a, 8, 32] AP over real cols of super-rows [a, b)."""
    ap = t2d[:, base + a * SR: base + b * SR]
    ap = ap.rearrange("p (r i c) -> p r i c", i=NIMG, c=NCOL)
    return ap[:, :, :, :32]


def build_program():
    nc = bacc.Bacc("TRN2", target_bir_lowering=False, debug=False,
                   num_devices=NCORES)

    # x strips laid out [p, half, c] so one DMA per pair covers both
    # halves.  bf16 copy feeds the sign path (sign-exact, half the bytes
    # on the startup-critical window); f32 only feeds the residual adds.
    xs = nc.declare_dram_parameter("xs", [128, 2, STRIP], F32, isOutput=False)
    xh = nc.declare_dram_parameter("xh", [128, 2, STRIP], BF16,
                                   isOutput=False)
    w1 = nc.declare_dram_parameter("w1", [128, 18, 2, 128], BF16,
                                   isOutput=False)
    w2 = nc.declare_dram_parameter("w2", [128, 18, 2, 128], BF16,
                                   isOutput=False)
    pv = nc.declare_dram_parameter("pv", [128, 2 * NPARAM], F32, isOutput=False)
    # out laid out [p, m, c] so one per-pair DMA covers both channel halves
    outd = nc.declare_dram_parameter("out", [128, 2, STRIP], BF16,
                                     isOutput=True)

    with tile.TileContext(nc) as tc:
        with (
            tc.tile_pool(name="big", bufs=1) as big,
            tc.tile_pool(name="wstage", bufs=2) as wsp,
            tc.tile_pool(name="w2stage", bufs=2) as w2sp,
            tc.tile_pool(name="xw", bufs=8) as xwp,
            tc.tile_pool(name="xs16", bufs=4) as xsp,
            tc.tile_pool(name="t2", bufs=4) as t2p,
            tc.tile_pool(name="psum", bufs=8, space="PSUM") as psp,
        ):
            s1t = [big.tile([128, 2, L], F8, tag=f"s1t{i}", name=f"s1t{i}")
                   for i, L in enumerate(S1LENS)]
            s2t = [big.tile([128, 2, L], F8, tag=f"s2t{i}", name=f"s2t{i}")
                   for i, L in enumerate(S2LENS)]
            o1p = [big.tile([128, 2, p[2]], F32, tag=f"o1p{k}",
                            name=f"o1p{k}")
                   for k, p in enumerate(PAIRS)]
            w1f = [big.tile([128, 9, 2, 128], F8, tag=f"w1f{m}",
                            name=f"w1f{m}") for m in range(2)]
            w2f = [big.tile([128, 9, 2, 128], F8, tag=f"w2f{m}",
                            name=f"w2f{m}") for m in range(2)]
            pt = big.tile([128, 2 * NPARAM], F32, tag="pt")
            dv = big.tile([128, 14], F32, tag="dv")
            scr = big.tile([128, 8], F32, tag="scr")
            scr2 = big.tile([128, 8], F32, tag="scr2")

            def w1ap(m, tap):
                return w1f[m][:, tap, :, :]

            def w2ap(m, tap):
                return w2f[m][:, tap, :, :]

            # dummy ACTIVATE with no data deps: forces the ACT table load
            # to happen immediately instead of before the first real evac
            nc.scalar.activation(scr2[:, 6:7],
                                 nc.const_aps.tensor(0.0, (128, 1)),
                                 AFT.Prelu, alpha=0.1)

            # warm-up operand memset FIRST on DVE (nonzero fp8 so the PE
            # datapath toggles -- zero x zero might not unthrottle HAM)
            U32 = mybir.dt.uint32
            win = big.tile([128, 256], F8, tag="warm_in", name="warm_in")
            nc.vector.memset(win[:, :], 1.0)

            # s1 guard zeroing on DVE (interior is fully overwritten by
            # sign1, pad cols included: host strip has zero pads,
            # sign(0)=0)
            for i, st in enumerate(s1t):
                dlen = (S1CUTS[i][1] - S1CUTS[i][0]) * SR
                nc.vector.memset(st[:, :, 0:GF].bitcast(U32), 0)
                nc.vector.memset(st[:, :, GF + dlen:S1LENS[i]], 0.0)

            # ---- x pair feeds ----------------------------------------
            # sign feed (GpSimd ring): one bf16 DMA per pair -> ScalarE
            # Sign into the s1 strip (contiguous writes keep the range
            # tracker exact; strided writes coarsen deps and stall PE).
            # residual feed (Sync ring, otherwise idle until the stores):
            # one f32 DMA per pair, needed only at the pair's add.
            xwt = {}
            signed = set()

            def feed_sign(pi, split=False):
                if pi in signed or pi >= len(PAIRS):
                    return
                signed.add(pi)
                _wis, e0, ln, r0, r1 = PAIRS[pi]
                xb = xsp.tile([128, 2, 3 * SR], BF16, tag="xh", name="xh")
                # split=True: per-half DMAs on BOTH rings so the halves
                # transfer in parallel and their ~2us completion receipts
                # pipeline -- worth it only for the startup-critical pairs
                for i in range(2):
                    if split:
                        eng = nc.gpsimd if i == 0 else nc.sync
                        eng.dma_start(out=xb[:, i:i + 1, :ln],
                                      in_=xh[:, i:i + 1, e0:e0 + ln])
                    elif i == 0:
                        nc.gpsimd.dma_start(out=xb[:, :, :ln],
                                            in_=xh[:, :, e0:e0 + ln])
                    for t, lo, hi in route(S1CUTS, r0, r1):
                        o0 = GF + (lo - S1CUTS[t][0]) * SR
                        nc.scalar.sign(
                            out=s1t[t][:, i, o0: o0 + (hi - lo) * SR],
                            in_=xb[:, i, (lo - r0) * SR:(hi - r0) * SR])

            def feed_res(pi):
                if pi in xwt or pi >= len(PAIRS):
                    return
                _wis, e0, ln, r0, r1 = PAIRS[pi]
                xb = xwp.tile([128, 2, 3 * SR], F32, tag="xw", name="xw")
                nc.gpsimd.dma_start(out=xb[:, :, :ln],
                                    in_=xs[:, :, e0:e0 + ln])
                xwt[pi] = xb

            # conv1 weight staging: {taps 0-4} then {taps 5-8} per half,
            # interleaved with the pair feeds on the one GpSimd ring in
            # need-order; VectorE binarize to (w>0)-0.5 = +-0.5
            w1s = []
            for m in range(2):
                w1s.append(wsp.tile([128, 9, 2, 128], BF16, tag="ws",
                                    name="ws"))

            def w1_dma(m, a, b):
                nc.gpsimd.dma_start(out=w1s[m][:, a:b, :, :],
                                    in_=w1[:, m * 9 + a:m * 9 + b, :, :])

            def w1_bin(m, a, b):
                nc.vector.tensor_scalar(w1f[m][:, a:b, :, :],
                                        w1s[m][:, a:b, :, :], 0.0, 0.5,
                                        AOP.is_gt, AOP.subtract)

            # params table first on the ring (11KB; it gates every evac
            # through the dv columns), then pair 0 + conv1 weights
            nc.sync.dma_start(out=pt[:, :], in_=pv[:, :])
            feed_sign(0, split=True)
            w1_dma(0, 0, 5)
            w1_bin(0, 0, 5)

            # PE warm-up: HAM re-clocks the array up only after ~5us of
            # sustained matmul activity; stream dummy matmuls while the
            # first x / weight DMAs land so the first real matmuls run at
            # the warm 2.4 GHz rate, not 1.2
            wps = psp.tile([128, WCOLS], F32, tag="ps", name="ps")
            for _ in range(30):
                nc.tensor.matmul(wps[:, :256], win[:, 0:128],
                                 win[:, 0:256], start=True, stop=True)

            feed_sign(1, split=True)
            w1_dma(0, 5, 9)
            w1_bin(0, 5, 9)
            w1_dma(1, 0, 5)
            w1_bin(1, 0, 5)

            def pcol(m, name):
                k = PARAM_ORDER.index(name)
                return pt[:, m * NPARAM + k: m * NPARAM + k + 1]

            def dcol(j):
                return dv[:, j: j + 1]

            # Batched rsqrt(var+eps) for all 4 (conv, half) columns at
            # once: Quake-III bit-trick seed + 3 Newton iterations, all on
            # VectorE (no ScalarE Sqrt -> no extra ACT table load).
            vco = [("bn1", 0), ("bn1", 1), ("bn2", 0), ("bn2", 1)]
            vpe = scr[:, 0:4]
            for j, (pfx, m) in enumerate(vco):
                nc.vector.tensor_scalar_add(scr[:, j:j + 1],
                                            pcol(m, pfx + "_var"), EPS)
            yb = scr[:, 4:8]
            nc.vector.memset(yb.bitcast(U32), 0x5f3759df)
            nc.vector.tensor_scalar(scr2[:, 0:4].bitcast(U32),
                                    vpe.bitcast(U32), 1, None,
                                    AOP.logical_shift_right)
            nc.vector.tensor_tensor(yb.bitcast(U32), yb.bitcast(U32),
                                    scr2[:, 0:4].bitcast(U32), AOP.subtract)
            for _ in range(3):
                nc.vector.tensor_tensor(scr2[:, 0:4], yb, yb, AOP.mult)
                nc.vector.tensor_tensor(scr2[:, 0:4], vpe, scr2[:, 0:4],
                                        AOP.mult)
                nc.vector.tensor_scalar(scr2[:, 0:4], scr2[:, 0:4], -0.5, 1.5,
                                        AOP.mult, AOP.add)
                nc.vector.tensor_tensor(yb, yb, scr2[:, 0:4], AOP.mult)

            # dv columns: per conv c (0/1), half m: sc = c*6+m, g*b = c*6+2+m,
            # d = c*6+4+m.  conv1 scale x2 (acts +-1 via ScalarE Sign,
            # weights +-0.5); conv2 scale x4 (acts +-0.5 via the DVE
            # is_gt sign2 trick, weights +-0.5).
            for j, (pfx, m) in enumerate(vco):
                ci = j // 2
                gam = pcol(m, pfx + "_gamma")
                bet = pcol(m, pfx + "_beta")
                mean = pcol(m, pfx + "_mean")
                bvec = pcol(m, "b1" if ci == 0 else "b2")
                rs = yb[:, j:j + 1]
                g = scr2[:, 4:5]
                nc.vector.tensor_tensor(g, gam, rs, AOP.mult)
                nc.vector.tensor_scalar_mul(dcol(ci * 6 + m), g,
                                            STD * (2.0, 4.0)[ci])
                nc.vector.tensor_tensor(dcol(ci * 6 + 2 + m), g, bvec, AOP.mult)
                nc.vector.tensor_tensor(scr2[:, 5:6], mean, g, AOP.mult)
                nc.vector.tensor_tensor(dcol(ci * 6 + 4 + m), bet,
                                        scr2[:, 5:6], AOP.subtract)

            # conv1 weights m=1 tail taps + pairs 2,3 signs behind the
            # params on the DVE queue; residual f32 for pair 0 follows on
            # the DMA ring
            w1_dma(1, 5, 9)
            w1_bin(1, 5, 9)
            feed_sign(2)
            feed_sign(3)
            feed_res(0)

            def sc_ap(conv, m):
                return dcol((conv - 1) * 6 + m)

            def bi_ap(conv, m):
                return dcol((conv - 1) * 6 + 2 + m)

            def dd_ap(conv, m):
                return dcol((conv - 1) * 6 + 4 + m)

            def al_ap(m):
                return pcol(m, "alpha")

            # s2 / w2 prep is deferred: only what conv1 group 0's sign2
            # writes (rows 0-3 -> s2t[0]) needs goes on the DVE queue now;
            # the rest is interleaved into the conv1 loop so it never
            # stalls group 0's PReLU chain behind a weight-DMA wait
            def memset_s2(i):
                for half in range(2):
                    nc.vector.memset(s2t[i][:, half, :].bitcast(U32), 0)

            def stage_w2(m):
                # 3-tap chunks so a late-landing DMA never head-blocks the
                # DVE queue for long; the first chunk's staging tile is
                # dep-pinned behind o1p[0] (written ~20us in) so the
                # scheduler cannot hoist w2 bytes into the startup window
                for c in range(3):
                    ws = w2sp.tile([128, 3, 2, 128], BF16, tag="w2s",
                                   name="w2s")
                    if m == 0 and c <= 1:
                        nc.vector.tensor_copy(ws[:, 0, 0, 0:1],
                                              o1p[0][:, 0, 0:1])
                    a, b = m * 9 + 3 * c, m * 9 + 3 * (c + 1)
                    nc.sync.dma_start(out=ws[:, :, :, :], in_=w2[:, a:b, :, :])
                    nc.vector.tensor_scalar(w2f[m][:, 3 * c:3 * (c + 1), :, :],
                                            ws[:, :, :, :], 0.0, 0.5,
                                            AOP.is_gt, AOP.subtract)

            memset_s2(0)

            # ---- the two convs: matmuls issued in 2-pair tap blocks so
            # each LDWEIGHTS covers 4 matmuls (hides the weight load);
            # post-ops stay per-pair (independent dependency chains) ------
            def conv_super(stiles, cuts, w2t, wap, convno, prs):
                first = convno == 1
                for m in range(2):
                    ps = {}
                    for pr in prs:
                        for wi in pr[0]:
                            ps[wi] = psp.tile([128, WCOLS], F32,
                                              tag="ps", name="ps")
                    for tap in range(9):
                        dy, dx = divmod(tap, 3)
                        off = (dy - 1) * SR + (dx - 1)
                        lhsT = wap(m, tap)
                        for pr in prs:
                            for wi in pr[0]:
                                we0, wln = WINDOWS[wi]
                                t = w2t[wi]
                                st = stiles[t]
                                c0 = GF + (we0 - cuts[t][0] * SR) + off
                                nc.tensor.matmul(
                                    ps[wi][:, :wln], lhsT,
                                    st[:, :, c0: c0 + wln],
                                    start=(tap == 0), stop=(tap == 8),
                                    perf_mode=DR)
                    for pr in prs:
                        wis, e0, ln, r0, r1 = pr
                        pi = e0 // (3 * SR)
                        if first:
                            # evac fuses BN scale + conv bias + PReLU in
                            # one ACT op (PReLU commutes with the positive
                            # gamma scale); DVE adds shift + residual
                            dst = o1p[pi][:, m, :ln]
                            o_off = 0
                            for wi in wis:
                                wln = WINDOWS[wi][1]
                                nc.scalar.activation(
                                    dst[:, o_off:o_off + wln],
                                    ps[wi][:, :wln], AFT.Prelu,
                                    bias=bi_ap(convno, m),
                                    scale=sc_ap(convno, m),
                                    alpha=al_ap(m))
                                o_off += wln
                            nc.vector.scalar_tensor_tensor(
                                dst, dst, dd_ap(convno, m),
                                xwt[pi][:, m, :ln], AOP.add, AOP.add)
                        else:
                            # conv2: per-window post-ops (shorter tail
                            # chain), both halves in one tile so the pair
                            # stores as a single [128,2,ln] DMA.  The
                            # residual add carries d2 (the BN2 shift) as
                            # its scalar operand: (t + d2) + o1.
                            if m == 0:
                                t2s[pi] = t2p.tile([128, 2, 3 * SR], BF16,
                                                   tag="t2", name="t2")
                            tb = t2s[pi]
                            o_off = 0
                            for wi in wis:
                                wln = WINDOWS[wi][1]
                                dw = tb[:, m, o_off:o_off + wln]
                                nc.scalar.activation(
                                    dw, ps[wi][:, :wln], AFT.Prelu,
                                    bias=bi_ap(convno, m),
                                    scale=sc_ap(convno, m),
                                    alpha=al_ap(m))
                                nc.vector.scalar_tensor_tensor(
                                    dw, dw, dd_ap(convno, m),
                                    o1p[pi][:, m, o_off:o_off + wln],
                                    AOP.add, AOP.add)
                                o_off += wln
                            if m == 1:
                                # stores alternate between the Sync HWDGE
                                # ring and the (idle by now) GpSimd SWDGE
                                # ring so the final drains run in parallel
                                eng = nc.sync if pi % 2 == 0 else nc.gpsimd
                                eng.dma_start(out=outd[:, :, e0:e0 + ln],
                                              in_=tb[:, :, :ln])
                if first:
                    # sign(out1) as (o1>0)-0.5 = +-0.5 on DVE (keeps the
                    # busy ScalarE FIFO out of the chain; the 2x is folded
                    # into conv2's evac scale), interior rows only
                    for pr in prs:
                        wis, e0, ln, r0, r1 = pr
                        pi = e0 // (3 * SR)
                        for m in range(2):
                            for t, lo, hi in route(S2CUTS, r0, r1):
                                base = S2CUTS[t][0]
                                nc.vector.tensor_scalar(
                                    _rows_ap(s2t[t][:, m], lo - base,
                                             hi - base, base=GF),
                                    _rows_ap(o1p[pi][:, m], lo - r0, hi - r0),
                                    0.0, 0.5, AOP.is_gt, AOP.subtract)

            t2s = {}
            # singleton first and last groups: the first matmuls gate only
            # on pair 0, and the final post-op/store chain covers only the
            # short 2-row pair
            sgroups = ([[PAIRS[0]]]
                       + [PAIRS[g:g + 2] for g in range(1, 9, 2)]
                       + [[PAIRS[9]], [PAIRS[10]]])
            # feeds / deferred s2+w2 prep interleaved behind each group
            a_sign = {0: [4, 5], 1: [6, 7], 2: [8, 9], 3: [10]}
            a_res = {0: [1, 2], 1: [3, 4], 2: [5, 6], 3: [7, 8], 4: [9, 10]}
            for gi, prs in enumerate(sgroups):
                conv_super(s1t, S1CUTS, W2T_1, w1ap, 1, prs)
                for pi in a_sign.get(gi, []):
                    feed_sign(pi)
                for pi in a_res.get(gi, []):
                    feed_res(pi)
                if gi == 1:
                    memset_s2(1)
                    stage_w2(0)
                elif gi == 2:
                    memset_s2(2)
                    stage_w2(1)
            for prs in sgroups:
                conv_super(s2t, S2CUTS, W2T_2, w2ap, 2, prs)

    nc.compile()
    return nc


# ---------------------------------------------------------------- host side

def _host_pack_x(x_shard):
    """[8,256,32,32] f32 -> strip layout [128,2,STRIP] with zero pad cols."""
    xz = np.zeros((128, 2, NSR, NIMG, NCOL), dtype=np.float32)
    xr = x_shard.reshape(NIMG, 2, 128, 32, 32)
    xz[:, :, :, :, :32] = xr.transpose(2, 1, 3, 0, 4)
    return np.ascontiguousarray(xz.reshape(128, 2, STRIP))


def _host_pack_xh(xz):
    """bf16 cast of the f32 strip (sign-exact for normal floats)."""
    import ml_dtypes
    return np.ascontiguousarray(xz.astype(ml_dtypes.bfloat16))


def _host_pack_w(W):
    """[256,256,3,3] -> [128(k), 18(m*9+tap), 2(i), 128(j)] bf16.

    bf16 preserves the sign of every f32 exactly (same exponent range, no
    flush to zero), and the kernel only uses sign(w)."""
    import ml_dtypes
    A = np.asarray(W, dtype=np.float32).reshape(2, 128, 2, 128, 3, 3)
    L = A.transpose(3, 4, 5, 0, 2, 1)          # (k, dy, dx, m, i, j)
    L = L.reshape(128, 9, 2, 2, 128)           # (k, tap, m, i, j)
    L = L.transpose(0, 2, 1, 3, 4)             # (k, m, tap, i, j)
    return np.ascontiguousarray(L.reshape(128, 18, 2, 128)
                                .astype(ml_dtypes.bfloat16))


def _host_pack_pv(inputs):
    pvt = np.zeros((128, 2 * NPARAM), dtype=np.float32)
    for k, name in enumerate(PARAM_ORDER):
        v = np.asarray(inputs[name], dtype=np.float32)
        for m in range(2):
            pvt[:, m * NPARAM + k] = v[m * 128:(m + 1) * 128]
    return pvt


def _host_unpack_out(o):
    """[128,2,STRIP] bf16 -> [8,256,32,32] f32."""
    o = np.asarray(o, dtype=np.float32)
    o = o.reshape(128, 2, NSR, NIMG, NCOL)[:, :, :, :, :32]
    return np.ascontiguousarray(o.transpose(3, 1, 0, 2, 4)
                                .reshape(NIMG, 256, 32, 32))


_PROG = None
LAST_EXEC_TIME_NS = None
LAST_RESULT = None


def _get_prog():
    global _PROG
    if _PROG is None:
        _PROG = build_program()
    return _PROG


def kernel(x, W1, b1, W2, b2, alpha,
           bn1_gamma, bn1_beta, bn1_mean, bn1_var,
           bn2_gamma, bn2_beta, bn2_mean, bn2_var,
           _trace=False):
    global LAST_EXEC_TIME_NS
    global LAST_RESULT
    inputs = dict(b1=b1, b2=b2, alpha=alpha,
                  bn1_gamma=bn1_gamma, bn1_beta=bn1_beta,
                  bn1_mean=bn1_mean, bn1_var=bn1_var,
                  bn2_gamma=bn2_gamma, bn2_beta=bn2_beta,
                  bn2_mean=bn2_mean, bn2_var=bn2_var)
    x = np.asarray(x, dtype=np.float32)
    w1l = _host_pack_w(W1)
    w2l = _host_pack_w(W2)
    pvt = _host_pack_pv(inputs)

    in_maps = []
    for c in range(NCORES):
        shard = x[c * NIMG:(c + 1) * NIMG]
        xz = _host_pack_x(shard)
        in_maps.append({"xs": xz, "xh": _host_pack_xh(xz), "w1": w1l,
                        "w2": w2l, "pv": pvt})

    nc = _get_prog()
    res = run_bass_kernel_spmd(nc, in_maps, core_ids=list(range(NCORES)),
                               trace=_trace)
    LAST_EXEC_TIME_NS = res.exec_time_ns
    LAST_RESULT = res

    outs = [_host_unpack_out(res.results[c]["out"]) for c in range(NCORES)]
    return np.concatenate(outs, axis=0)



# revision 43
# speedup vs baseline: 1.0259x; 1.0259x over previous
"""Trainium2 Bass kernel for a binarized (1w1a) BasicBlock:

    out = BN2(PReLU(conv3x3(sign(x1), std2*sign(W2)) + b2)) + x1
    x1  = BN1(PReLU(conv3x3(sign(x),  std1*sign(W1)) + b1)) + x

Strategy
--------
Data-parallel over the batch axis: each of the 8 NeuronCores processes 8 of
the 64 images, with the (small) weights / BN / PReLU params replicated.
No collectives are needed.

Per-core compute:
  * Activations and weights are binarized (ScalarE Sign -> {-1,0,+1} for
    conv1's input; the (x>0)-0.5 = +-0.5 VectorE trick for weights and for
    sign(out1)), exactly representable in fp8e4m3.  The 3x3 conv over
    256->256 channels is 9 shifted matmuls accumulating in PSUM, using the
    fp8 DoubleRow perf mode so each matmul contracts the full K=256 input
    channels (2 fp8 weights per PE cell).  The scale factors (x2 conv1,
    x4 conv2) are folded into the BN scale of the PSUM evacuation.
  * Pad-row-free strip layout: per partition, super-row r (r = 0..31)
    concatenates row r of all 8 images, each 33 wide (32 real + 1 zero pad
    col); 32 super-rows of 264 form one 8448-element strip per channel
    half, with zero guards on both ends.  Vertical 3x3 taps are +-264,
    horizontal +-1, so every tap is a constant offset and a 396-element
    window (<= 1 PSUM bank) is one uniform-stride matmul.
  * The PE stream (~131us) is the spine; everything else hides behind it:
    - PE warm-up: dummy matmuls on an all-ones fp8 tile while the first
      DMAs land, so HAM un-throttles (1.2 -> 2.4 GHz) before the first
      real matmul instead of ~5us into the stream.
    - PSUM evacuation is ONE ScalarE op: AFT.Prelu with per-channel alpha
      fuses conv-bias + BN scale + PReLU (PReLU commutes with the
      positive gamma scale).  DVE then does one fused add (+ BN shift
      via the scalar operand, + residual) per window.
    - sign(out1) runs on DVE as (o1>0)-0.5 (ScalarE's FIFO is the
      contended resource early on; the 2x folds into conv2's scale).
    - x feeds: a host-cast bf16 copy of x feeds the sign path (half the
      startup-critical bytes; bf16 is sign-exact), the f32 copy loads
      later for the residual adds only.  One DMA per pair, params table
      first on the ring, w1 in {taps 0-4}/{taps 5-8} chunks interleaved
      at need-order ring slots.  w2's staging is dep-pinned behind o1p[0]
      so the (ready-list) Tile scheduler cannot hoist its bytes into the
      startup window, and 3-tap-chunked so a late DMA never head-blocks
      the DVE queue.
    - conv groups: singleton first group (first matmuls gate only on pair
      0) and singleton last groups (short final post-op chain); per-pair
      [128,2,ln] output stores alternate between the Sync HWDGE ring and
      the (idle by then) GpSimd SWDGE ring.
  * Writes into the s1/s2 strips stay contiguous-per-half where matmuls
    must not over-wait: the Tile range tracker coarsens strided writes to
    whole-tile deps, which serializes the PE behind unrelated sign ops.

The host side only reshapes/transposes/zero-pads/casts (layout), shards the
batch and un-packs the output strip.  All arithmetic (sign, BN folding,
conv, PReLU, residual) happens on-device.
"""

import math
import os
import sys

import numpy as np

for _p in ("/opt/trn_rl_repo", "/root/.axon_site/_ro/trn_rl_repo"):
    if os.path.isdir(_p) and _p not in sys.path:
        sys.path.insert(0, _p)

import concourse.bass as bass
import concourse.bacc as bacc
import concourse.mybir as mybir
from concourse import tile
from concourse.bass_utils import run_bass_kernel_spmd

F32 = mybir.dt.float32
BF16 = mybir.dt.bfloat16
F8 = mybir.dt.float8e4
AOP = mybir.AluOpType
AFT = mybir.ActivationFunctionType
DR = mybir.MatmulPerfMode.DoubleRow

EPS = 1e-5
NCORES = 8
NIMG = 8            # images per core
NCOL = 33           # cols per image row (32 real + 1 pad)
SR = NIMG * NCOL    # super-row length: row r of all 8 images  (264)
NSR = 32            # super-rows per strip
STRIP = NSR * SR    # 8448
GF = 272            # front guard (>= SR + 1, 16-aligned)
WCOLS = 396         # window: 1.5 super-rows (<= 512 PSUM bank)
STD = math.sqrt(2.0) / math.sqrt(256 * 9)

# strip tile splits, in super-rows (see module docstring)
S1CUTS = [(0, 18), (15, 32)]
S2CUTS = [(0, 18), (15, 25), (23, 32)]
W2T_1 = [0] * 11 + [1] * 11              # window -> s1 tile
W2T_2 = [0] * 11 + [1] * 5 + [2] * 6     # window -> s2 tile


def _slen(cut):
    n = GF + (cut[1] - cut[0]) * SR + 288
    return ((n + 15) // 16) * 16


S1LENS = [_slen(c) for c in S1CUTS]
S2LENS = [_slen(c) for c in S2CUTS]


def route(cuts, ra, rb):
    """Map super-row range [ra, rb) onto the strip tiles."""
    out = []
    for t, (lo, hi) in enumerate(cuts):
        a, b = max(ra, lo), min(rb, hi)
        if a < b:
            out.append((t, a, b))
    return out


# windows: (e0, ln); 21 full 396-col windows + one 132-col tail
WINDOWS = []
_e = 0
while _e < STRIP:
    ln = min(WCOLS, STRIP - _e)
    WINDOWS.append((_e, ln))
    _e += ln

# pairs: 2 windows each -> super-row aligned (3 rows; last pair 2 rows)
PAIRS = []
for _i in range(0, len(WINDOWS), 2):
    ws_ = WINDOWS[_i:_i + 2]
    e0 = ws_[0][0]
    ln = sum(w[1] for w in ws_)
    assert e0 % SR == 0 and ln % SR == 0
    PAIRS.append((list(range(_i, _i + len(ws_))), e0, ln,
                  e0 // SR, (e0 + ln) // SR))

# per-channel param column order inside the packed [128, 22] table
PARAM_ORDER = [
    "b1", "alpha", "bn1_gamma", "bn1_beta", "bn1_mean", "bn1_var",
    "b2", "bn2_gamma", "bn2_beta", "bn2_mean", "bn2_var",
]
NPARAM = len(PARAM_ORDER)


def _rows_ap(t2d, a, b, base=0):
    """[128, b-a, 8, 32] AP over real cols of super-rows [a, b)."""
    ap = t2d[:, base + a * SR: base + b * SR]
    ap = ap.rearrange("p (r i c) -> p r i c", i=NIMG, c=NCOL)
    return ap[:, :, :, :32]


def build_program():
    nc = bacc.Bacc("TRN2", target_bir_lowering=False, debug=False,
                   num_devices=NCORES)

    # x strips laid out [p, half, c] so one DMA per pair covers both
    # halves.  bf16 copy feeds the sign path (sign-exact, half the bytes
    # on the startup-critical window); f32 only feeds the residual adds.
    xs = nc.declare_dram_parameter("xs", [128, 2, STRIP], F32, isOutput=False)
    xh = nc.declare_dram_parameter("xh", [128, 2, STRIP], BF16,
                                   isOutput=False)
    w1 = nc.declare_dram_parameter("w1", [128, 18, 2, 128], BF16,
                                   isOutput=False)
    w2 = nc.declare_dram_parameter("w2", [128, 18, 2, 128], BF16,
                                   isOutput=False)
    pv = nc.declare_dram_parameter("pv", [128, 2 * NPARAM], F32, isOutput=False)
    # out laid out [p, m, c] so one per-pair DMA covers both channel halves
    outd = nc.declare_dram_parameter("out", [128, 2, STRIP], BF16,
                                     isOutput=True)

    with tile.TileContext(nc) as tc:
        with (
            tc.tile_pool(name="big", bufs=1) as big,
            tc.tile_pool(name="wstage", bufs=2) as wsp,
            tc.tile_pool(name="w2stage", bufs=2) as w2sp,
            tc.tile_pool(name="xw", bufs=8) as xwp,
            tc.tile_pool(name="xs16", bufs=4) as xsp,
            tc.tile_pool(name="t2", bufs=4) as t2p,
            tc.tile_pool(name="psum", bufs=8, space="PSUM") as psp,
        ):
            s1t = [big.tile([128, 2, L], F8, tag=f"s1t{i}", name=f"s1t{i}")
                   for i, L in enumerate(S1LENS)]
            s2t = [big.tile([128, 2, L], F8, tag=f"s2t{i}", name=f"s2t{i}")
                   for i, L in enumerate(S2LENS)]
            o1p = [big.tile([128, 2, p[2]], F32, tag=f"o1p{k}",
                            name=f"o1p{k}")
                   for k, p in enumerate(PAIRS)]
            w1f = [big.tile([128, 9, 2, 128], F8, tag=f"w1f{m}",
                            name=f"w1f{m}") for m in range(2)]
            w2f = [big.tile([128, 9, 2, 128], F8, tag=f"w2f{m}",
                            name=f"w2f{m}") for m in range(2)]
            pt = big.tile([128, 2 * NPARAM], F32, tag="pt")
            dv = big.tile([128, 14], F32, tag="dv")
            scr = big.tile([128, 8], F32, tag="scr")
            scr2 = big.tile([128, 8], F32, tag="scr2")

            def w1ap(m, tap):
                return w1f[m][:, tap, :, :]

            def w2ap(m, tap):
                return w2f[m][:, tap, :, :]

            # dummy ACTIVATE with no data deps: forces the ACT table load
            # to happen immediately instead of before the first real evac
            nc.scalar.activation(scr2[:, 6:7],
                                 nc.const_aps.tensor(0.0, (128, 1)),
                                 AFT.Prelu, alpha=0.1)

            # warm-up operand memset FIRST on DVE (nonzero fp8 so the PE
            # datapath toggles -- zero x zero might not unthrottle HAM)
            U32 = mybir.dt.uint32
            win = big.tile([128, 256], F8, tag="warm_in", name="warm_in")
            nc.vector.memset(win[:, :], 1.0)

            # s1 guard zeroing on DVE (interior is fully overwritten by
            # sign1, pad cols included: host strip has zero pads,
            # sign(0)=0)
            for i, st in enumerate(s1t):
                dlen = (S1CUTS[i][1] - S1CUTS[i][0]) * SR
                nc.vector.memset(st[:, :, 0:GF].bitcast(U32), 0)
                nc.vector.memset(st[:, :, GF + dlen:S1LENS[i]], 0.0)

            # ---- x pair feeds ----------------------------------------
            # sign feed (GpSimd ring): one bf16 DMA per pair -> ScalarE
            # Sign into the s1 strip (contiguous writes keep the range
            # tracker exact; strided writes coarsen deps and stall PE).
            # residual feed (Sync ring, otherwise idle until the stores):
            # one f32 DMA per pair, needed only at the pair's add.
            xwt = {}
            signed = set()

            def feed_sign(pi, split=False):
                if pi in signed or pi >= len(PAIRS):
                    return
                signed.add(pi)
                _wis, e0, ln, r0, r1 = PAIRS[pi]
                xb = xsp.tile([128, 2, 3 * SR], BF16, tag="xh", name="xh")
                # split=True: per-half DMAs so half 0's sign overlaps half
                # 1's transfer (the ~2us completion receipts pipeline) --
                # worth it only for the startup-critical pair 0
                for i in range(2):
                    if split:
                        nc.gpsimd.dma_start(out=xb[:, i:i + 1, :ln],
                                            in_=xh[:, i:i + 1, e0:e0 + ln])
                    elif i == 0:
                        nc.gpsimd.dma_start(out=xb[:, :, :ln],
                                            in_=xh[:, :, e0:e0 + ln])
                    for t, lo, hi in route(S1CUTS, r0, r1):
                        o0 = GF + (lo - S1CUTS[t][0]) * SR
                        nc.scalar.sign(
                            out=s1t[t][:, i, o0: o0 + (hi - lo) * SR],
                            in_=xb[:, i, (lo - r0) * SR:(hi - r0) * SR])

            def feed_res(pi):
                if pi in xwt or pi >= len(PAIRS):
                    return
                _wis, e0, ln, r0, r1 = PAIRS[pi]
                xb = xwp.tile([128, 2, 3 * SR], F32, tag="xw", name="xw")
                nc.gpsimd.dma_start(out=xb[:, :, :ln],
                                    in_=xs[:, :, e0:e0 + ln])
                xwt[pi] = xb

            # conv1 weight staging: {taps 0-4} then {taps 5-8} per half,
            # interleaved with the pair feeds on the one GpSimd ring in
            # need-order; VectorE binarize to (w>0)-0.5 = +-0.5
            w1s = []
            for m in range(2):
                w1s.append(wsp.tile([128, 9, 2, 128], BF16, tag="ws",
                                    name="ws"))

            def w1_dma(m, a, b):
                nc.gpsimd.dma_start(out=w1s[m][:, a:b, :, :],
                                    in_=w1[:, m * 9 + a:m * 9 + b, :, :])

            def w1_bin(m, a, b):
                nc.vector.tensor_scalar(w1f[m][:, a:b, :, :],
                                        w1s[m][:, a:b, :, :], 0.0, 0.5,
                                        AOP.is_gt, AOP.subtract)

            # params table first on the ring (11KB; it gates every evac
            # through the dv columns), then pair 0 + conv1 weights
            nc.sync.dma_start(out=pt[:, :], in_=pv[:, :])
            feed_sign(0, split=True)
            w1_dma(0, 0, 5)
            w1_bin(0, 0, 5)

            # PE warm-up: HAM re-clocks the array up only after ~5us of
            # sustained matmul activity; stream dummy matmuls while the
            # first x / weight DMAs land so the first real matmuls run at
            # the warm 2.4 GHz rate, not 1.2
            wps = psp.tile([128, WCOLS], F32, tag="ps", name="ps")
            for _ in range(30):
                nc.tensor.matmul(wps[:, :256], win[:, 0:128],
                                 win[:, 0:256], start=True, stop=True)

            feed_sign(1)
            w1_dma(0, 5, 9)
            w1_bin(0, 5, 9)
            w1_dma(1, 0, 5)
            w1_bin(1, 0, 5)

            def pcol(m, name):
                k = PARAM_ORDER.index(name)
                return pt[:, m * NPARAM + k: m * NPARAM + k + 1]

            def dcol(j):
                return dv[:, j: j + 1]

            # Batched rsqrt(var+eps) for all 4 (conv, half) columns at
            # once: Quake-III bit-trick seed + 3 Newton iterations, all on
            # VectorE (no ScalarE Sqrt -> no extra ACT table load).
            vco = [("bn1", 0), ("bn1", 1), ("bn2", 0), ("bn2", 1)]
            vpe = scr[:, 0:4]
            for j, (pfx, m) in enumerate(vco):
                nc.vector.tensor_scalar_add(scr[:, j:j + 1],
                                            pcol(m, pfx + "_var"), EPS)
            yb = scr[:, 4:8]
            nc.vector.memset(yb.bitcast(U32), 0x5f3759df)
            nc.vector.tensor_scalar(scr2[:, 0:4].bitcast(U32),
                                    vpe.bitcast(U32), 1, None,
                                    AOP.logical_shift_right)
            nc.vector.tensor_tensor(yb.bitcast(U32), yb.bitcast(U32),
                                    scr2[:, 0:4].bitcast(U32), AOP.subtract)
            for _ in range(3):
                nc.vector.tensor_tensor(scr2[:, 0:4], yb, yb, AOP.mult)
                nc.vector.tensor_tensor(scr2[:, 0:4], vpe, scr2[:, 0:4],
                                        AOP.mult)
                nc.vector.tensor_scalar(scr2[:, 0:4], scr2[:, 0:4], -0.5, 1.5,
                                        AOP.mult, AOP.add)
                nc.vector.tensor_tensor(yb, yb, scr2[:, 0:4], AOP.mult)

            # dv columns: per conv c (0/1), half m: sc = c*6+m, g*b = c*6+2+m,
            # d = c*6+4+m.  conv1 scale x2 (acts +-1 via ScalarE Sign,
            # weights +-0.5); conv2 scale x4 (acts +-0.5 via the DVE
            # is_gt sign2 trick, weights +-0.5).
            for j, (pfx, m) in enumerate(vco):
                ci = j // 2
                gam = pcol(m, pfx + "_gamma")
                bet = pcol(m, pfx + "_beta")
                mean = pcol(m, pfx + "_mean")
                bvec = pcol(m, "b1" if ci == 0 else "b2")
                rs = yb[:, j:j + 1]
                g = scr2[:, 4:5]
                nc.vector.tensor_tensor(g, gam, rs, AOP.mult)
                nc.vector.tensor_scalar_mul(dcol(ci * 6 + m), g,
                                            STD * (2.0, 4.0)[ci])
                nc.vector.tensor_tensor(dcol(ci * 6 + 2 + m), g, bvec, AOP.mult)
                nc.vector.tensor_tensor(scr2[:, 5:6], mean, g, AOP.mult)
                nc.vector.tensor_tensor(dcol(ci * 6 + 4 + m), bet,
                                        scr2[:, 5:6], AOP.subtract)

            # conv1 weights m=1 tail taps + pairs 2,3 signs behind the
            # params on the DVE queue; residual f32 for pair 0 follows on
            # the DMA ring
            w1_dma(1, 5, 9)
            w1_bin(1, 5, 9)
            feed_sign(2)
            feed_sign(3)
            feed_res(0)

            def sc_ap(conv, m):
                return dcol((conv - 1) * 6 + m)

            def bi_ap(conv, m):
                return dcol((conv - 1) * 6 + 2 + m)

            def dd_ap(conv, m):
                return dcol((conv - 1) * 6 + 4 + m)

            def al_ap(m):
                return pcol(m, "alpha")

            # s2 / w2 prep is deferred: only what conv1 group 0's sign2
            # writes (rows 0-3 -> s2t[0]) needs goes on the DVE queue now;
            # the rest is interleaved into the conv1 loop so it never
            # stalls group 0's PReLU chain behind a weight-DMA wait
            def memset_s2(i):
                for half in range(2):
                    nc.vector.memset(s2t[i][:, half, :].bitcast(U32), 0)

            def stage_w2(m):
                # 3-tap chunks so a late-landing DMA never head-blocks the
                # DVE queue for long; the first chunk's staging tile is
                # dep-pinned behind o1p[0] (written ~20us in) so the
                # scheduler cannot hoist w2 bytes into the startup window
                for c in range(3):
                    ws = w2sp.tile([128, 3, 2, 128], BF16, tag="w2s",
                                   name="w2s")
                    if m == 0 and c <= 1:
                        nc.vector.tensor_copy(ws[:, 0, 0, 0:1],
                                              o1p[0][:, 0, 0:1])
                    a, b = m * 9 + 3 * c, m * 9 + 3 * (c + 1)
                    nc.sync.dma_start(out=ws[:, :, :, :], in_=w2[:, a:b, :, :])
                    nc.vector.tensor_scalar(w2f[m][:, 3 * c:3 * (c + 1), :, :],
                                            ws[:, :, :, :], 0.0, 0.5,
                                            AOP.is_gt, AOP.subtract)

            memset_s2(0)

            # ---- the two convs: matmuls issued in 2-pair tap blocks so
            # each LDWEIGHTS covers 4 matmuls (hides the weight load);
            # post-ops stay per-pair (independent dependency chains) ------
            def conv_super(stiles, cuts, w2t, wap, convno, prs):
                first = convno == 1
                for m in range(2):
                    ps = {}
                    for pr in prs:
                        for wi in pr[0]:
                            ps[wi] = psp.tile([128, WCOLS], F32,
                                              tag="ps", name="ps")
                    for tap in range(9):
                        dy, dx = divmod(tap, 3)
                        off = (dy - 1) * SR + (dx - 1)
                        lhsT = wap(m, tap)
                        for pr in prs:
                            for wi in pr[0]:
                                we0, wln = WINDOWS[wi]
                                t = w2t[wi]
                                st = stiles[t]
                                c0 = GF + (we0 - cuts[t][0] * SR) + off
                                nc.tensor.matmul(
                                    ps[wi][:, :wln], lhsT,
                                    st[:, :, c0: c0 + wln],
                                    start=(tap == 0), stop=(tap == 8),
                                    perf_mode=DR)
                    for pr in prs:
                        wis, e0, ln, r0, r1 = pr
                        pi = e0 // (3 * SR)
                        if first:
                            # evac fuses BN scale + conv bias + PReLU in
                            # one ACT op (PReLU commutes with the positive
                            # gamma scale); DVE adds shift + residual
                            dst = o1p[pi][:, m, :ln]
                            o_off = 0
                            for wi in wis:
                                wln = WINDOWS[wi][1]
                                nc.scalar.activation(
                                    dst[:, o_off:o_off + wln],
                                    ps[wi][:, :wln], AFT.Prelu,
                                    bias=bi_ap(convno, m),
                                    scale=sc_ap(convno, m),
                                    alpha=al_ap(m))
                                o_off += wln
                            nc.vector.scalar_tensor_tensor(
                                dst, dst, dd_ap(convno, m),
                                xwt[pi][:, m, :ln], AOP.add, AOP.add)
                        else:
                            # conv2: per-window post-ops (shorter tail
                            # chain), both halves in one tile so the pair
                            # stores as a single [128,2,ln] DMA.  The
                            # residual add carries d2 (the BN2 shift) as
                            # its scalar operand: (t + d2) + o1.
                            if m == 0:
                                t2s[pi] = t2p.tile([128, 2, 3 * SR], BF16,
                                                   tag="t2", name="t2")
                            tb = t2s[pi]
                            o_off = 0
                            for wi in wis:
                                wln = WINDOWS[wi][1]
                                dw = tb[:, m, o_off:o_off + wln]
                                nc.scalar.activation(
                                    dw, ps[wi][:, :wln], AFT.Prelu,
                                    bias=bi_ap(convno, m),
                                    scale=sc_ap(convno, m),
                                    alpha=al_ap(m))
                                nc.vector.scalar_tensor_tensor(
                                    dw, dw, dd_ap(convno, m),
                                    o1p[pi][:, m, o_off:o_off + wln],
                                    AOP.add, AOP.add)
                                o_off += wln
                            if m == 1:
                                # stores alternate between the Sync HWDGE
                                # ring and the (idle by now) GpSimd SWDGE
                                # ring so the final drains run in parallel
                                eng = nc.sync if pi % 2 == 0 else nc.gpsimd
                                eng.dma_start(out=outd[:, :, e0:e0 + ln],
                                              in_=tb[:, :, :ln])
                if first:
                    # sign(out1) as (o1>0)-0.5 = +-0.5 on DVE (keeps the
                    # busy ScalarE FIFO out of the chain; the 2x is folded
                    # into conv2's evac scale), interior rows only
                    for pr in prs:
                        wis, e0, ln, r0, r1 = pr
                        pi = e0 // (3 * SR)
                        for m in range(2):
                            for t, lo, hi in route(S2CUTS, r0, r1):
                                base = S2CUTS[t][0]
                                nc.vector.tensor_scalar(
                                    _rows_ap(s2t[t][:, m], lo - base,
                                             hi - base, base=GF),
                                    _rows_ap(o1p[pi][:, m], lo - r0, hi - r0),
                                    0.0, 0.5, AOP.is_gt, AOP.subtract)

            t2s = {}
            # singleton first and last groups: the first matmuls gate only
            # on pair 0, and the final post-op/store chain covers only the
            # short 2-row pair
            sgroups = ([[PAIRS[0]]]
                       + [PAIRS[g:g + 2] for g in range(1, 9, 2)]
                       + [[PAIRS[9]], [PAIRS[10]]])
            # feeds / deferred s2+w2 prep interleaved behind each group
            a_sign = {0: [4, 5], 1: [6, 7], 2: [8, 9], 3: [10]}
            a_res = {0: [1, 2], 1: [3, 4], 2: [5, 6], 3: [7, 8], 4: [9, 10]}
            for gi, prs in enumerate(sgroups):
                conv_super(s1t, S1CUTS, W2T_1, w1ap, 1, prs)
                for pi in a_sign.get(gi, []):
                    feed_sign(pi)
                for pi in a_res.get(gi, []):
                    feed_res(pi)
                if gi == 1:
                    memset_s2(1)
                    stage_w2(0)
                elif gi == 2:
                    memset_s2(2)
                    stage_w2(1)
            for prs in sgroups:
                conv_super(s2t, S2CUTS, W2T_2, w2ap, 2, prs)

    nc.compile()
    return nc


# ---------------------------------------------------------------- host side

def _host_pack_x(x_shard):
    """[8,256,32,32] f32 -> strip layout [128,2,STRIP] with zero pad cols."""
    xz = np.zeros((128, 2, NSR, NIMG, NCOL), dtype=np.float32)
    xr = x_shard.reshape(NIMG, 2, 128, 32, 32)
    xz[:, :, :, :, :32] = xr.transpose(2, 1, 3, 0, 4)
    return np.ascontiguousarray(xz.reshape(128, 2, STRIP))


def _host_pack_xh(xz):
    """bf16 cast of the f32 strip (sign-exact for normal floats)."""
    import ml_dtypes
    return np.ascontiguousarray(xz.astype(ml_dtypes.bfloat16))


def _host_pack_w(W):
    """[256,256,3,3] -> [128(k), 18(m*9+tap), 2(i), 128(j)] bf16.

    bf16 preserves the sign of every f32 exactly (same exponent range, no
    flush to zero), and the kernel only uses sign(w)."""
    import ml_dtypes
    A = np.asarray(W, dtype=np.float32).reshape(2, 128, 2, 128, 3, 3)
    L = A.transpose(3, 4, 5, 0, 2, 1)          # (k, dy, dx, m, i, j)
    L = L.reshape(128, 9, 2, 2, 128)           # (k, tap, m, i, j)
    L = L.transpose(0, 2, 1, 3, 4)             # (k, m, tap, i, j)
    return np.ascontiguousarray(L.reshape(128, 18, 2, 128)
                                .astype(ml_dtypes.bfloat16))


def _host_pack_pv(inputs):
    pvt = np.zeros((128, 2 * NPARAM), dtype=np.float32)
    for k, name in enumerate(PARAM_ORDER):
        v = np.asarray(inputs[name], dtype=np.float32)
        for m in range(2):
            pvt[:, m * NPARAM + k] = v[m * 128:(m + 1) * 128]
    return pvt


def _host_unpack_out(o):
    """[128,2,STRIP] bf16 -> [8,256,32,32] f32."""
    o = np.asarray(o, dtype=np.float32)
    o = o.reshape(128, 2, NSR, NIMG, NCOL)[:, :, :, :, :32]
    return np.ascontiguousarray(o.transpose(3, 1, 0, 2, 4)
                                .reshape(NIMG, 256, 32, 32))


_PROG = None
LAST_EXEC_TIME_NS = None
LAST_RESULT = None


def _get_prog():
    global _PROG
    if _PROG is None:
        _PROG = build_program()
    return _PROG


def kernel(x, W1, b1, W2, b2, alpha,
           bn1_gamma, bn1_beta, bn1_mean, bn1_var,
           bn2_gamma, bn2_beta, bn2_mean, bn2_var,
           _trace=False):
    global LAST_EXEC_TIME_NS
    global LAST_RESULT
    inputs = dict(b1=b1, b2=b2, alpha=alpha,
                  bn1_gamma=bn1_gamma, bn1_beta=bn1_beta,
                  bn1_mean=bn1_mean, bn1_var=bn1_var,
                  bn2_gamma=bn2_gamma, bn2_beta=bn2_beta,
                  bn2_mean=bn2_mean, bn2_var=bn2_var)
    x = np.asarray(x, dtype=np.float32)
    w1l = _host_pack_w(W1)
    w2l = _host_pack_w(W2)
    pvt = _host_pack_pv(inputs)

    in_maps = []
    for c in range(NCORES):
        shard = x[c * NIMG:(c + 1) * NIMG]
        xz = _host_pack_x(shard)
        in_maps.append({"xs": xz, "xh": _host_pack_xh(xz), "w1": w1l,
                        "w2": w2l, "pv": pvt})

    nc = _get_prog()
    res = run_bass_kernel_spmd(nc, in_maps, core_ids=list(range(NCORES)),
                               trace=_trace)
    LAST_EXEC_TIME_NS = res.exec_time_ns
    LAST_RESULT = res

    outs = [_host_unpack_out(res.results[c]["out"]) for c in range(NCORES)]
    return np.concatenate(outs, axis=0)

